# revision 3
# baseline (speedup 1.0000x reference)
"""Causal self-attention on 8 trn2 NeuronCores.

Sharding: the 32 (batch, head) pairs are split head-wise — core c owns heads
{2c, 2c+1} for both batches (perfectly causal-balanced, no cross-core skew).
Each core computes QKV for its heads over the full sequence (column-parallel
c_attn), runs attention, then an AllToAll exchanges head-channels for token
slices so each core applies the full output projection (row contraction over
all heads) to its own 512-token slice. Output is re-assembled host-side.

All matmuls run in float32r (full-rate fp32 mode on the PE, ~1.5e-4 rel err).
"""

import sys

sys.path.insert(0, "/opt/trn_rl_repo")

import numpy as np

import concourse.bass as bass
import concourse.mybir as mybir
import concourse.tile as tile
from concourse.bass_utils import run_bass_kernel_spmd

N_CORES = 8
B, T, C = 2, 2048, 2048
NH, HD = 16, 128
P = 128
KC = C // P            # 16 contraction subtiles
NB = 4                 # 512-wide t-chunks per batch
QC = 4                 # 512-wide q-chunks per batch
HL = 2                 # heads per core
BT = B * T             # 4096
TSL = BT // N_CORES    # 512 output tokens per core

f32 = mybir.dt.float32
f32r = mybir.dt.float32r
ACTF = mybir.ActivationFunctionType
ALU = mybir.AluOpType

_CACHE = {}


def _split_multi_waits(nc, max_waits=1):
    """This container's walrus rejects >1 sync-wait per instruction; hoist
    extra waits onto same-engine NoOps placed just before the instruction."""
    n_split = 0
    for fn in nc.m.functions:
        for bb in fn.blocks:
            insts = list(bb.instructions)
            out = []
            changed = False
            for inst in insts:
                si = inst.sync_info
                waits = list(si.on_wait) if (si is not None and si.on_wait) else []
                if len(waits) > max_waits:
                    ups = list(si.on_update) if si.on_update else []
                    head, tail = waits[:-max_waits], waits[-max_waits:]
                    for i, w in enumerate(head):
                        nop = mybir.InstNoOp(name=f"{inst.name}-wsplit-{i}")
                        nop.engine = inst.engine
                        nop.sync_info = mybir.SyncInfo(on_wait=[w], on_update=[])
                        out.append(nop)
                    inst.sync_info = mybir.SyncInfo(on_wait=tail, on_update=ups)
                    changed = True
                    n_split += 1
                out.append(inst)
            if changed:
                bb.instructions = out
    return n_split


def _build_bass():
    nc = bass.Bass("TRN2", target_bir_lowering=False, debug=False,
                   num_devices=N_CORES)

    xT = nc.declare_dram_parameter("xT", [C, BT], f32, isOutput=False)
    w_qkv = nc.declare_dram_parameter("w_qkv", [C, 3 * HL * HD], f32,
                                      isOutput=False)
    b_qkv = nc.declare_dram_parameter("b_qkv", [3 * HL * HD], f32,
                                      isOutput=False)
    w_proj = nc.declare_dram_parameter("w_proj", [C, C], f32, isOutput=False)
    b_proj = nc.declare_dram_parameter("b_proj", [C], f32, isOutput=False)
    dmask = nc.declare_dram_parameter("dmask", [4, P, 512], f32,
                                      isOutput=False)
    ones_m = nc.declare_dram_parameter("ones_m", [P, P], f32, isOutput=False)
    ones_c = nc.declare_dram_parameter("ones_c", [1, P], f32, isOutput=False)
    outT = nc.declare_dram_parameter("outT", [C, TSL], f32, isOutput=True)

    xT_t = xT.rearrange("(kc p) t -> p kc t", p=P)          # [128,16,4096]
    wq_t = w_qkv.rearrange("(kc p) n -> p kc n", p=P)       # [128,16,768]
    wp_t = w_proj.rearrange("(kc p) n -> p kc n", p=P)      # [128,16,2048]

    scale = float(HD) ** -0.5

    with tile.TileContext(nc) as tc:
        with (
            tc.tile_pool(name="const", bufs=1) as cpool,
            tc.tile_pool(name="dram", bufs=1, space="DRAM") as dram,
        ):
            # Constants resident for the whole kernel
            wq_sb = cpool.tile([P, KC, 3 * HL * HD], f32r)
            nc.sync.dma_start(wq_sb[:], wq_t.bitcast(f32r))
            bqk_sb = cpool.tile([P, 4], f32)                 # q/k bias per col
            nc.sync.dma_start(
                bqk_sb[:], b_qkv[0:2 * HL * HD].rearrange("(m p) -> p m", p=P))
            bv_sb = cpool.tile([1, HL * HD], f32r)           # v bias row
            nc.sync.dma_start(
                bv_sb[:], b_qkv[2 * HL * HD:3 * HL * HD].bitcast(f32r)[None, :])
            bp_sb = cpool.tile([P, KC], f32)                 # proj bias
            nc.sync.dma_start(bp_sb[:],
                              b_proj.rearrange("(m p) -> p m", p=P))
            dm_sb = cpool.tile([P, 4, 512], f32r)            # diag masks
            nc.sync.dma_start(dm_sb[:],
                              dmask.rearrange("d k c -> k d c").bitcast(f32r))
            onesm_sb = cpool.tile([P, P], f32r)
            nc.sync.dma_start(onesm_sb[:], ones_m[:, :].bitcast(f32r))
            onesc_sb = cpool.tile([1, P], f32r)
            nc.sync.dma_start(onesc_sb[:], ones_c[:, :].bitcast(f32r))

            a2a_in = dram.tile([N_CORES, HL * HD, TSL], f32)
            a2a_out = dram.tile([N_CORES, HL * HD, TSL], f32)

            for b in range(B):
                with tc.tile_pool(name=f"qkv_b{b}", bufs=1) as bpool:
                    qk_sb = bpool.tile([P, 4, T], f32r)      # qh0 qh1 kh0 kh1
                    v_sb = bpool.tile([P, KC, HL * HD], f32r)

                    # ---- QKV projection for this batch ----
                    with (
                        tc.tile_pool(name="xin", bufs=2) as xpool,
                        tc.tile_pool(name="qk_ps", bufs=2, space="PSUM") as qkps,
                        tc.tile_pool(name="v_ps", bufs=2, space="PSUM") as vps,
                    ):
                        for nb in range(NB):
                            g = b * T + nb * 512
                            xc = xpool.tile([P, KC, 512], f32r)
                            nc.sync.dma_start(
                                xc[:], xT_t[:, :, g:g + 512].bitcast(f32r))
                            for m in range(4):               # qh0 qh1 kh0 kh1
                                ps = qkps.tile([P, 512], f32)
                                for kc in range(KC):
                                    nc.tensor.matmul(
                                        ps[:],
                                        wq_sb[:, kc, m * P:(m + 1) * P],
                                        xc[:, kc, :],
                                        start=(kc == 0), stop=(kc == KC - 1))
                                nc.scalar.activation(
                                    out=qk_sb[:, m, nb * 512:(nb + 1) * 512],
                                    in_=ps[:], func=ACTF.Identity,
                                    bias=bqk_sb[:, m:m + 1], scale=1.0)
                            for tv in range(4):
                                ps = vps.tile([P, HL * HD], f32)
                                for kc in range(KC):
                                    nc.tensor.matmul(
                                        ps[:],
                                        xc[:, kc, tv * P:(tv + 1) * P],
                                        wq_sb[:, kc, 2 * HL * HD:3 * HL * HD],
                                        start=(kc == 0), stop=False)
                                nc.tensor.matmul(
                                    ps[:], onesc_sb[:], bv_sb[:],
                                    start=False, stop=True)
                                nc.vector.tensor_copy(
                                    v_sb[:, nb * 4 + tv, :], ps[:])

                    # ---- attention for this batch's two heads ----
                    with (
                        tc.tile_pool(name="s_ps", bufs=2, space="PSUM") as sps,
                        tc.tile_pool(name="o_ps", bufs=2, space="PSUM") as ops,
                        tc.tile_pool(name="r_ps", bufs=2, space="PSUM") as rps,
                        tc.tile_pool(name="probs", bufs=3) as ppool,
                        tc.tile_pool(name="att_ev", bufs=2) as aev,
                    ):
                        for hl in range(HL):
                            qT_h = qk_sb[:, hl]              # [128, 2048]
                            kT_h = qk_sb[:, 2 + hl]
                            for qc in range(QC):
                                o_ps = ops.tile([P, 512], f32)
                                r_ps = rps.tile([P, 512], f32)
                                nkb = 4 * qc + 4
                                for kb in range(nkb):
                                    s_ps = sps.tile([P, 512], f32)
                                    nc.tensor.matmul(
                                        s_ps[:],
                                        kT_h[:, kb * P:(kb + 1) * P],
                                        qT_h[:, qc * 512:(qc + 1) * 512],
                                        start=True, stop=True)
                                    probs = ppool.tile([P, 512], f32r)
                                    nc.scalar.activation(
                                        out=probs[:], in_=s_ps[:],
                                        func=ACTF.Exp, scale=scale)
                                    dq = kb - 4 * qc
                                    if dq >= 0:              # diagonal region
                                        nc.vector.tensor_tensor(
                                            out=probs[:], in0=probs[:],
                                            in1=dm_sb[:, dq, :], op=ALU.mult)
                                    nc.tensor.matmul(
                                        o_ps[:],
                                        v_sb[:, kb, hl * HD:(hl + 1) * HD],
                                        probs[:],
                                        start=(kb == 0), stop=(kb == nkb - 1))
                                    nc.tensor.matmul(
                                        r_ps[:], onesm_sb[:], probs[:],
                                        start=(kb == 0), stop=(kb == nkb - 1))
                                recip = aev.tile([P, 512], f32, tag="recip")
                                nc.vector.reciprocal(recip[:], r_ps[:])
                                o_sb = aev.tile([P, 512], f32, tag="osb")
                                nc.vector.tensor_tensor(
                                    out=o_sb[:], in0=o_ps[:], in1=recip[:],
                                    op=ALU.mult)
                                nc.sync.dma_start(
                                    a2a_in[b * 4 + qc,
                                           hl * HD:(hl + 1) * HD, :],
                                    o_sb[:])

            # ---- head exchange ----
            nc.gpsimd.collective_compute(
                "AllToAll", ALU.bypass,
                replica_groups=[list(range(N_CORES))],
                ins=[a2a_in.opt()], outs=[a2a_out.opt()])

            # ---- output projection on own token slice ----
            with (
                tc.tile_pool(name="prhs", bufs=1) as prhs,
                tc.tile_pool(name="pw", bufs=2) as pw,
                tc.tile_pool(name="p_ps", bufs=2, space="PSUM") as pps,
                tc.tile_pool(name="pout", bufs=2) as pout,
            ):
                rhs_sb = prhs.tile([P, KC, TSL], f32r)
                a2a_flat = a2a_out.rearrange("j r t -> (j r) t")
                nc.sync.dma_start(
                    rhs_sb[:],
                    a2a_flat.rearrange("(kc p) t -> p kc t", p=P).bitcast(f32r))
                for m in range(KC):
                    w_sb = pw.tile([P, KC, P], f32r)
                    nc.sync.dma_start(
                        w_sb[:], wp_t[:, :, m * P:(m + 1) * P].bitcast(f32r))
                    ps = pps.tile([P, TSL], f32)
                    for kc in range(KC):
                        nc.tensor.matmul(ps[:], w_sb[:, kc, :],
                                         rhs_sb[:, kc, :],
                                         start=(kc == 0), stop=(kc == KC - 1))
                    o = pout.tile([P, TSL], f32)
                    nc.scalar.activation(out=o[:], in_=ps[:], func=ACTF.Identity,
                                         bias=bp_sb[:, m:m + 1], scale=1.0)
                    nc.sync.dma_start(outT[m * P:(m + 1) * P, :], o[:])

    _split_multi_waits(nc)
    return nc


def _host_inputs(x, w_attn, b_attn, w_proj, b_proj):
    x = np.ascontiguousarray(np.asarray(x, dtype=np.float32))
    w_attn = np.ascontiguousarray(np.asarray(w_attn, dtype=np.float32))
    b_attn = np.ascontiguousarray(np.asarray(b_attn, dtype=np.float32))
    w_proj = np.ascontiguousarray(np.asarray(w_proj, dtype=np.float32))
    b_proj = np.ascontiguousarray(np.asarray(b_proj, dtype=np.float32))

    xT = np.ascontiguousarray(x.reshape(BT, C).T)

    dmask = np.zeros((4, P, 512), dtype=np.float32)
    cols = np.arange(512)
    ks = np.arange(P)
    for dq in range(4):
        dmask[dq] = (cols[None, :] - dq * P >= ks[:, None]).astype(np.float32)

    ones_m = np.ones((P, P), dtype=np.float32)
    ones_c = np.ones((1, P), dtype=np.float32)

    in_maps = []
    for c in range(N_CORES):
        h0 = HL * c
        col = h0 * HD
        w_qkv = np.concatenate(
            [w_attn[:, col:col + HL * HD],
             w_attn[:, C + col:C + col + HL * HD],
             w_attn[:, 2 * C + col:2 * C + col + HL * HD]], axis=1)
        b_qkv = np.concatenate(
            [b_attn[col:col + HL * HD],
             b_attn[C + col:C + col + HL * HD],
             b_attn[2 * C + col:2 * C + col + HL * HD]])
        in_maps.append({
            "xT": xT,
            "w_qkv": np.ascontiguousarray(w_qkv),
            "b_qkv": np.ascontiguousarray(b_qkv),
            "w_proj": w_proj,
            "b_proj": b_proj,
            "dmask": dmask,
            "ones_m": ones_m,
            "ones_c": ones_c,
        })
    return in_maps


def kernel(x, w_attn, b_attn, w_proj, b_proj, _results_out=None):
    if "nc" not in _CACHE:
        _CACHE["nc"] = _build_bass()
    nc = _CACHE["nc"]
    in_maps = _host_inputs(x, w_attn, b_attn, w_proj, b_proj)
    res = run_bass_kernel_spmd(nc, in_maps, list(range(N_CORES)))
    if _results_out is not None:
        _results_out.append(res)
    outT = np.concatenate([res.results[c]["outT"] for c in range(N_CORES)],
                          axis=1)                            # [C, B*T]
    return np.ascontiguousarray(outT.T).reshape(B, T, C)
